# revision 1
# baseline (speedup 1.0000x reference)
"""Trainium2 Bass kernel for nn_Ballistics (v2: bf16 + 3-engine split).

Per-batch attack/release one-pole: y[t] = (1-c)*y[t-1] + c*x[t],
c = at if x[t] > y[t-1] else rt.  Margin form u[j] = y[j-1] - x[j]:
    u[j+1] = a_j*u[j] - dx[j+1],  a_j = a_rt if u[j] >= 0 else a_at.
Predicate fixed-point iteration (K rounds), each round solving the
linearized recurrence exactly with the native tensor_tensor_scan.

Changes vs the original fp32 baseline (381.8us):
  - all streams bf16 (scan state stays fp32 internally) -> TT 2x, TS 4x
  - W=128 warm-up (a_max^128 = 2.4e-6)
  - K=5 rounds (verified rel err 1.14e-2 vs 2e-2 budget on the real inputs)
  - engine split: scans on DVE (6) + GPSIMD (14), predicates on DVE
    (bf16 tensor_scalar 4x), affine b->a on ScalarE activation
  - all 4 groups in flight (tile pools bufs=4)
"""
import sys
for p in ("/opt/trn_rl_repo", "/root/.axon_site/_ro/trn_rl_repo"):
    if p not in sys.path:
        sys.path.insert(0, p)

import os
import numpy as np

B, T = 64, 262144
NCORES = 8
RPC = B // NCORES          # rows per core (8)
L = 4096                   # chunk length (output steps per chunk)
W = 128                    # warm-up steps (discarded)
K = 5                      # predicate iterations
C = T // L                 # chunks per row (64)
QP = 128 // RPC            # chunks in flight per row (16)
G = C // QP                # sequential groups (4)
N = L + W + 1              # scan steps per window
NW = N + 1                 # x-window columns per partition

BF = True                  # bf16 streams (x, dx, att, u, b, y)
# Scans are DVE-only (walrus rejects TensorTensorScan on Pool).  The pred+aff
# pairs per (g,k) slot go to ScalarE (Sign-form, both ops) or to GPSIMD-pred +
# DVE-aff (is_ge form), spread round-robin.  dx and final add placement via env.
N_ACT_PREDS = int(os.environ.get("BALL_NPRED_ACT", "15"))  # rest on DVE
N_ACT_AFFS = int(os.environ.get("BALL_NAFF_ACT", "15"))    # rest on DVE
DX_GP = os.environ.get("BALL_DXGP", "0") == "1"
ADD_GP = os.environ.get("BALL_ADDGP", "0") == "1"
ORDER = os.environ.get("BALL_ORDER", "stage")

def _spread(n, total=20):
    K_, G_ = 5, 4
    slots = [(g, k) for k in range(K_) for g in range(G_)]
    if n <= 0:
        return set()
    step = total / n
    return {slots[min(int(i * step), total - 1)] for i in range(n)}

# k=0 seed predicates write an odd bf16 column (bt[:, 1:NW]) which drops DVE
# to 1x mode on real HW; ScalarE is 1x regardless, so seeds always go there.
ACT_PRED_SLOTS = {(g, 0) for g in range(4)} | _spread(max(0, N_ACT_PREDS - 4))
ACT_AFF_SLOTS = _spread(N_ACT_AFFS)
PIPE = G                   # groups in flight

_cache = {}


def _build(reps=1):
    import concourse.bacc as bacc
    import concourse.mybir as mybir
    import concourse.tile as tile
    import concourse.bass as bass

    f32 = mybir.dt.float32
    DT = mybir.dt.bfloat16 if BF else f32
    Alu = mybir.AluOpType
    Act = mybir.ActivationFunctionType

    nc = bacc.Bacc("TRN2", target_bir_lowering=False, debug=False,
                   num_devices=NCORES)
    x_d = nc.dram_tensor("x", [RPC, T], DT, kind="ExternalInput")
    aat_d = nc.dram_tensor("aat", [128, 1], f32, kind="ExternalInput")
    dlt_d = nc.dram_tensor("dlt", [128, 1], f32, kind="ExternalInput")
    mid_d = nc.dram_tensor("mid", [128, 1], f32, kind="ExternalInput")
    hdl_d = nc.dram_tensor("hdl", [128, 1], f32, kind="ExternalInput")
    y_d = nc.dram_tensor("y", [RPC, T], DT, kind="ExternalOutput")

    with tile.TileContext(nc) as tc:
        with tc.tile_pool(name="cpool", bufs=1) as cpool, \
             tc.tile_pool(name="xpool", bufs=PIPE) as xpool, \
             tc.tile_pool(name="ypool", bufs=4) as ypool, \
             tc.tile_pool(name="wpool", bufs=PIPE) as wpool:
            aat_s = cpool.tile([128, 1], f32, tag="aat")
            dlt_s = cpool.tile([128, 1], f32, tag="dlt")
            mid_s = cpool.tile([128, 1], f32, tag="mid")
            hdl_s = cpool.tile([128, 1], f32, tag="hdl")
            nc.sync.dma_start(aat_s[:, :], aat_d.ap()[:, :])
            nc.sync.dma_start(dlt_s[:, :], dlt_d.ap()[:, :])
            nc.sync.dma_start(mid_s[:, :], mid_d.ap()[:, :])
            nc.sync.dma_start(hdl_s[:, :], hdl_d.ap()[:, :])

            def start_group(gr):
                """DMA-in + dx + init; returns group state dict."""
                xt = xpool.tile([128, NW], DT, tag="xt")
                base = gr * QP * L - W - 1
                if gr == 0:
                    # chunk 0 (q=0): pad cols [0, W+1) with 1.0, then x[0:L+1].
                    nc.vector.memset(xt[:, 0:W + 1], 1.0)
                    for r in range(RPC):
                        p0 = r * QP
                        nc.sync.dma_start(
                            xt[p0:p0 + 1, W + 1:NW],
                            bass.AP(x_d, r * T, [[1, L + 1]]))
                        nc.sync.dma_start(
                            xt[p0 + 1:p0 + QP, :],
                            bass.AP(x_d, r * T + L - W - 1,
                                    [[L, QP - 1], [1, NW]]))
                elif gr == G - 1:
                    # last chunk (q=15): col NW-1 would be x[T] -> pad 0.0
                    nc.vector.memset(xt[:, NW - 1:NW], 0.0)
                    for r in range(RPC):
                        p0 = r * QP
                        nc.sync.dma_start(
                            xt[p0:p0 + QP - 1, :],
                            bass.AP(x_d, r * T + base, [[L, QP - 1], [1, NW]]))
                        nc.sync.dma_start(
                            xt[p0 + QP - 1:p0 + QP, 0:NW - 1],
                            bass.AP(x_d, r * T + base + (QP - 1) * L,
                                    [[1, NW - 1]]))
                else:
                    for r in range(RPC):
                        nc.sync.dma_start(
                            xt[r * QP:(r + 1) * QP, :],
                            bass.AP(x_d, r * T + base, [[L, QP], [1, NW]]))

                dxt = wpool.tile([128, N], DT, tag="dx")
                e = nc.gpsimd if DX_GP else nc.vector
                e.tensor_tensor(dxt[:, :], xt[:, 1:NW], xt[:, 0:N],
                                Alu.subtract)
                ut = wpool.tile([128, NW], DT, tag="ut")
                att = wpool.tile([128, NW], DT, tag="att")
                bt = wpool.tile([128, NW], DT, tag="bt")
                nc.vector.memset(ut[:, 0:1], 0.0)
                # (no att memset: the affine writes att[:, :] before any read)
                nc.vector.memset(bt[:, 0:1], 0.0)
                return dict(gr=gr, xt=xt, dxt=dxt, ut=ut, att=att, bt=bt)

            def emit_pred(st, k):
                dxt, ut, bt = st["dxt"], st["ut"], st["bt"]
                if (st["gr"], k) in ACT_PRED_SLOTS:
                    # Sign-form on ScalarE: s in {-1,0,1}
                    if k == 0:
                        nc.scalar.activation(bt[:, 1:NW], dxt[:, 0:N],
                                             Act.Sign, scale=-1.0)
                    else:
                        nc.scalar.activation(bt[:, :], ut[:, :], Act.Sign)
                else:
                    # is_ge form on DVE: b in {0,1}  (GPSIMD is ~25x slower
                    # on real HW than the cost model claims -- never use it)
                    if k == 0:
                        nc.vector.tensor_single_scalar(
                            bt[:, 1:NW], dxt[:, 0:N], 0.0, Alu.is_le)
                    else:
                        nc.vector.tensor_single_scalar(
                            bt[:, :], ut[:, :], 0.0, Alu.is_ge)

            def emit_aff(st, k):
                att, bt = st["att"], st["bt"]
                # scalar pair must match the pred's output form
                if (st["gr"], k) in ACT_PRED_SLOTS:
                    sc, bi = hdl_s, mid_s          # att = mid + hdl*s
                else:
                    sc, bi = dlt_s, aat_s          # att = aat + dlt*b
                if (st["gr"], k) in ACT_AFF_SLOTS:
                    nc.scalar.activation(att[:, :], bt[:, :], Act.Identity,
                                         bias=bi[:, 0:1],
                                         scale=sc[:, 0:1])
                else:
                    nc.vector.tensor_scalar(
                        att[:, :], bt[:, :], sc[:, 0:1],
                        bi[:, 0:1], Alu.mult, Alu.add)

            def emit_scan(st, k):
                nc.vector.tensor_tensor_scan(
                    st["ut"][:, 1:NW], st["att"][:, 0:N], st["dxt"][:, :], 0.0,
                    Alu.mult, Alu.subtract)

            def finish_group(st):
                gr, xt, ut = st["gr"], st["xt"], st["ut"]
                yt = ypool.tile([128, L], DT, tag="yt")
                e = nc.gpsimd if ADD_GP else nc.vector
                e.tensor_tensor(
                    yt[:, :], xt[:, W + 2:W + 2 + L],
                    ut[:, W + 2:W + 2 + L], Alu.add)
                nc.sync.dma_start(
                    bass.AP(y_d, gr * QP * L, [[T, RPC], [L, QP], [1, L]]),
                    yt[:, :])

            for _rep in range(reps):
                sts = [start_group(gr) for gr in range(G)]
                for k in range(K):
                    if ORDER == "group":
                        for st in sts:
                            emit_pred(st, k)
                            emit_aff(st, k)
                            emit_scan(st, k)
                    else:
                        for st in sts:
                            emit_pred(st, k)
                        for st in sts:
                            emit_aff(st, k)
                        for st in sts:
                            emit_scan(st, k)
                for st in sts:
                    finish_group(st)

    nc.compile()
    return nc


def _get_nc(reps=1):
    if reps not in _cache:
        _cache[reps] = _build(reps)
    return _cache[reps]


def _coeffs(z_alpha):
    z = np.asarray(z_alpha, dtype=np.float32)
    ts = (np.float32(1.0) / (np.float32(1.0) + np.exp(-z, dtype=np.float32)))
    at = ts[:, 0].astype(np.float32)
    rt = ts[:, 1].astype(np.float32)
    a_at = (np.float32(1.0) - at).astype(np.float32)
    a_rt = (np.float32(1.0) - rt).astype(np.float32)
    return a_at, a_rt


def _in_maps(signal, z_alpha):
    import ml_dtypes
    a_at, a_rt = _coeffs(z_alpha)
    dlt = (a_rt - a_at).astype(np.float32)
    mid = ((a_at + a_rt) * np.float32(0.5)).astype(np.float32)
    hdl = ((a_rt - a_at) * np.float32(0.5)).astype(np.float32)
    xdt = ml_dtypes.bfloat16 if BF else np.float32
    sig = np.ascontiguousarray(np.asarray(signal, dtype=np.float32)).astype(xdt)
    prow = np.arange(128) // QP
    maps = []
    for ci in range(NCORES):
        rows = slice(ci * RPC, (ci + 1) * RPC)
        sel = ci * RPC + prow
        maps.append({
            "x": sig[rows],
            "aat": a_at[sel][:, None].astype(np.float32),
            "dlt": dlt[sel][:, None].astype(np.float32),
            "mid": mid[sel][:, None].astype(np.float32),
            "hdl": hdl[sel][:, None].astype(np.float32),
        })
    return maps


def kernel(signal, z_alpha):
    from concourse import bass_utils
    nc = _get_nc()
    maps = _in_maps(signal, z_alpha)
    res = bass_utils.run_bass_kernel_spmd(nc, maps, core_ids=list(range(NCORES)))
    out = np.concatenate([np.asarray(r["y"], dtype=np.float32)
                          for r in res.results], axis=0)
    return out


if __name__ == "__main__":
    rng = np.random.default_rng(0)
    sig = rng.standard_normal((B, T)).astype(np.float32)
    za = rng.standard_normal((B, 2)).astype(np.float32)
    y = kernel(sig, za)
    print("kernel ran:", y.shape, y.dtype)

